# revision 2
# baseline (speedup 1.0000x reference)
"""Trainium2 Bass kernel for CaputoEncoder — fused on-chip pipeline (v2).

Structure per core (pure batch parallelism, PB=8 batches, both LSTM layers):
  For_i over 17 windows of 32 timesteps. Body(iw):
    1. caputo feats for window iw   (PE matmuls, G^T window DMA'd from DRAM)
    2. xw0(iw) = A0 @ feats + b0    (PE + Pool bias-copies, SBUF-resident)
    3. interleaved scans: L0 window iw  <->  L1 window iw-1
       (each step: Pool preloads xw slice into PSUM, 64 accumulate-matmuls,
        Act sigmoid/tanh straight off PSUM, fused [i|f]*[g|c] elementwise)
    4. xw1(iw) = A1 @ h0win + b1    (consumed by L1 next iteration)
  Everything lives in SBUF; the only per-window DMA is the 32-col G^T slice.

Key tricks:
  * caputo(x, 1.0) == 0 (1/gamma(0)=0) -> only alpha=0.5 branch matters.
  * xw joined to gates via PSUM preload (Pool copy) + start=False matmuls,
    removing a DVE hop from the recurrence critical path.
  * L1 lags one window; zero xw window preserves zero state exactly, so the
    prologue memset of xw1 stands in for the missing first window.
  * [i|f] x [g|c] fused into one DVE op via the X-state tile layout.
"""

import math
from contextlib import ExitStack

import numpy as np
import ml_dtypes

import concourse.bass as bass
import concourse.tile as tile
from concourse import mybir
from concourse.bass import ds
from concourse.bass_utils import run_bass_kernel_spmd

AF = mybir.ActivationFunctionType
OP = mybir.AluOpType
F32 = mybir.dt.float32
BF16 = mybir.dt.bfloat16

B, T, N = 64, 512, 250
H = 512
G4 = 4 * H
OUT = 1024
NCORES = 8
PB = B // NCORES          # 8 batches per core
WIN = 32                  # timesteps per window
NWIN = T // WIN           # 16 real windows
NITER = NWIN + 1          # +1 window of L1 lag
TPAD = NITER * WIN        # 544 G^T columns
KC = H // 128             # 4 hidden chunks
GC = G4 // 128            # 16 gate chunks
MC = 2                    # 2 input-feature chunks (250 -> 256)
NP = MC * 128
CB = KC * PB              # 32 h/c columns


def _split_drain_waits(nc, max_waits=1):
    """CoreV3 codegen accepts one sem-wait per engine instruction; spill the
    rest onto same-engine NoOps immediately before it."""
    for bb in nc.m.functions[0].blocks:
        insts = bb.instructions
        i = 0
        while i < len(insts):
            ins = insts[i]
            si = ins.sync_info
            if si is not None and len(si.on_wait) > max_waits:
                waits = list(si.on_wait)
                ins.sync_info = mybir.SyncInfo(
                    on_wait=waits[:max_waits], on_update=list(si.on_update)
                )
                for j, w in enumerate(waits[max_waits:]):
                    nop = mybir.InstNoOp(name=f"{ins.name}-wsplit{j}")
                    nop.engine = ins.engine
                    nop.sync_info = mybir.SyncInfo(on_wait=[w], on_update=[])
                    insts.insert(i, nop)
                    i += 1
            i += 1


def _scan_step(nc, ps, whh_sb, h_aps, X_cur, X_next, PO, Y, TC, h_dst):
    """One LSTM step for one layer. ps is the gates PSUM tile (preloaded with
    xw); h_aps is a list of 4 (128, PB) APs for h_{t-1} chunks."""
    for gc in range(GC):
        for kc in range(KC):
            nc.tensor.matmul(
                ps[:, gc * PB:(gc + 1) * PB],
                whh_sb[:, kc, gc * 128:(gc + 1) * 128],
                h_aps[kc],
                start=False,
                stop=(kc == KC - 1),
                skip_group_check=True,
            )
    # gates perm [i,f,o,g]: i cols 0:32, f 32:64, o 64:96, g 96:128
    nc.scalar.activation(PO[:], ps[:, :3 * CB], AF.Sigmoid)
    nc.scalar.activation(X_cur[:, :CB], ps[:, 3 * CB:], AF.Tanh)
    # Y = [i|f] * [g|c]
    nc.vector.tensor_tensor(Y[:], PO[:, :2 * CB], X_cur[:], OP.mult)
    nc.vector.tensor_tensor(
        X_next[:, CB:], Y[:, :CB], Y[:, CB:], OP.add
    )  # c_new
    nc.scalar.activation(TC[:], X_next[:, CB:], AF.Tanh)
    nc.vector.tensor_tensor(h_dst, PO[:, 2 * CB:], TC[:], OP.mult)


def build_nc():
    nc = bass.Bass()

    x_in = nc.dram_tensor("xt", [PB, KC, 128, NP], BF16, kind="ExternalInput")
    gt_in = nc.dram_tensor("gt", [KC, 128, TPAD], BF16, kind="ExternalInput")
    a0_in = nc.dram_tensor("a0t", [MC, 128, G4], BF16, kind="ExternalInput")
    b0_in = nc.dram_tensor("b0", [128, GC], F32, kind="ExternalInput")
    whh0_in = nc.dram_tensor("whh0t", [KC, 128, G4], BF16, kind="ExternalInput")
    a1_in = nc.dram_tensor("a1t", [KC, 128, G4], BF16, kind="ExternalInput")
    b1_in = nc.dram_tensor("b1", [128, GC], F32, kind="ExternalInput")
    whh1_in = nc.dram_tensor("whh1t", [KC, 128, G4], BF16, kind="ExternalInput")
    wout_in = nc.dram_tensor("woutt", [KC, 128, OUT], BF16, kind="ExternalInput")
    bout_in = nc.dram_tensor("boutr", [PB, OUT], F32, kind="ExternalInput")
    out_ext = nc.dram_tensor("out", [PB, OUT], F32, kind="ExternalOutput")

    with tile.TileContext(nc) as tc:
        with ExitStack() as ctx:
            cpool = ctx.enter_context(tc.tile_pool(name="consts", bufs=1))
            spool = ctx.enter_context(tc.tile_pool(name="state", bufs=1))

            x_sb = cpool.tile([128, PB, KC, NP], BF16)
            nc.sync.dma_start(
                x_sb[:], x_in[:, :, :, :].rearrange("b k p n -> p b k n")
            )
            a0_sb = cpool.tile([128, MC, G4], BF16)
            nc.sync.dma_start(a0_sb[:], a0_in[:, :, :].rearrange("k p g -> p k g"))
            b0_sb = cpool.tile([128, GC], F32)
            nc.sync.dma_start(b0_sb[:], b0_in[:, :])
            whh0_sb = cpool.tile([128, KC, G4], BF16)
            nc.sync.dma_start(
                whh0_sb[:], whh0_in[:, :, :].rearrange("k p g -> p k g")
            )
            a1_sb = cpool.tile([128, KC, G4], BF16)
            nc.sync.dma_start(a1_sb[:], a1_in[:, :, :].rearrange("k p g -> p k g"))
            b1_sb = cpool.tile([128, GC], F32)
            nc.sync.dma_start(b1_sb[:], b1_in[:, :])
            whh1_sb = cpool.tile([128, KC, G4], BF16)
            nc.sync.dma_start(
                whh1_sb[:], whh1_in[:, :, :].rearrange("k p g -> p k g")
            )
            wout_sb = cpool.tile([128, KC, OUT], BF16)
            nc.sync.dma_start(
                wout_sb[:], wout_in[:, :, :].rearrange("k p g -> p k g")
            )
            bout_sb = cpool.tile([PB, OUT], F32)
            nc.sync.dma_start(bout_sb[:], bout_in[:, :])

            # persistent state
            xw0_sb = spool.tile([128, GC, WIN, PB], BF16)
            xw1_sb = spool.tile([128, GC, WIN, PB], BF16)
            hwin = spool.tile([128, KC, WIN, PB], BF16)
            feats = spool.tile([128, MC, WIN, PB], BF16)
            Hlast = spool.tile([128, CB], BF16)       # L0 h at window edge
            H1 = [spool.tile([128, CB], BF16, name=f"H1_{i}") for i in range(2)]
            X0 = [spool.tile([128, 2 * CB], F32, name=f"X0_{i}") for i in range(2)]
            X1 = [spool.tile([128, 2 * CB], F32, name=f"X1_{i}") for i in range(2)]
            nc.vector.memset(xw1_sb[:], 0.0)
            nc.vector.memset(Hlast[:], 0.0)
            for t_ in H1 + X0 + X1:
                nc.vector.memset(t_[:], 0.0)
            nc.vector.memset(hwin[:], 0.0)

            with tc.tile_pool(name="gtw", bufs=2) as gtw_pool, \
                 tc.tile_pool(name="tps", bufs=2, space="PSUM") as tps_pool, \
                 tc.tile_pool(name="ps0", bufs=2, space="PSUM") as ps0_pool, \
                 tc.tile_pool(name="ps1", bufs=2, space="PSUM") as ps1_pool, \
                 tc.tile_pool(name="ew", bufs=3) as ew_pool:
                with tc.For_i(0, NITER, 1,
                              hint_engines=(mybir.EngineType.PE,)) as iw:
                    # ---- caputo feats(iw): feats^T = x^T @ G^T window ----
                    gtw = gtw_pool.tile([128, KC, WIN], BF16, tag="gtw")
                    nc.sync.dma_start(
                        gtw[:], gt_in[:, :, ds(iw * WIN, WIN)].rearrange(
                            "k p w -> p k w"
                        ),
                    )
                    for b in range(PB):
                        for mc in range(MC):
                            psC = tps_pool.tile([128, WIN], F32, tag="psC")
                            for kc in range(KC):
                                nc.tensor.matmul(
                                    psC[:],
                                    x_sb[:, b, kc, mc * 128:(mc + 1) * 128],
                                    gtw[:, kc, :],
                                    start=(kc == 0),
                                    stop=(kc == KC - 1),
                                )
                            nc.scalar.activation(
                                feats[:, mc, :, b], psC[:], AF.Copy
                            )
                    # ---- xw0(iw) = A0 @ feats + b0 ----
                    for gc in range(GC):
                        px = tps_pool.tile([128, WIN * PB], F32, tag="px")
                        for mc in range(MC):
                            nc.tensor.matmul(
                                px[:],
                                a0_sb[:, mc, gc * 128:(gc + 1) * 128],
                                feats[:, mc].rearrange("p w b -> p (w b)"),
                                start=(mc == 0),
                                stop=(mc == MC - 1),
                            )
                        nc.vector.tensor_scalar_add(
                            xw0_sb[:, gc].rearrange("p w b -> p (w b)"),
                            px[:], b0_sb[:, gc:gc + 1],
                        )
                    # ---- interleaved scans: L0(iw) and L1(iw-1) ----
                    for u in range(WIN):
                        # L0
                        ps0 = ps0_pool.tile([128, GC * PB], F32, tag="ps0")
                        nc.scalar.copy(
                            ps0[:].rearrange("p (g b) -> p g b", g=GC),
                            xw0_sb[:, :, u, :],
                        )
                        h_aps = (
                            [Hlast[:, kc * PB:(kc + 1) * PB] for kc in range(KC)]
                            if u == 0 else
                            [hwin[:, kc, u - 1, :] for kc in range(KC)]
                        )
                        PO = ew_pool.tile([128, 3 * CB], F32, tag="PO0")
                        Y = ew_pool.tile([128, 2 * CB], F32, tag="Y0")
                        TC_ = ew_pool.tile([128, CB], F32, tag="TC0")
                        _scan_step(
                            nc, ps0, whh0_sb, h_aps,
                            X0[u % 2], X0[(u + 1) % 2], PO, Y, TC_,
                            hwin[:, :, u, :],
                        )
                        if u == WIN - 1:
                            nc.gpsimd.tensor_copy(
                                Hlast[:].rearrange("p (k b) -> p k b", k=KC),
                                hwin[:, :, u, :],
                            )
                        # L1 (window iw-1, xw1_sb filled last iteration)
                        ps1 = ps1_pool.tile([128, GC * PB], F32, tag="ps1")
                        nc.scalar.copy(
                            ps1[:].rearrange("p (g b) -> p g b", g=GC),
                            xw1_sb[:, :, u, :],
                        )
                        h_aps1 = [
                            H1[u % 2][:, kc * PB:(kc + 1) * PB]
                            for kc in range(KC)
                        ]
                        PO1 = ew_pool.tile([128, 3 * CB], F32, tag="PO1")
                        Y1 = ew_pool.tile([128, 2 * CB], F32, tag="Y1")
                        TC1 = ew_pool.tile([128, CB], F32, tag="TC1")
                        _scan_step(
                            nc, ps1, whh1_sb, h_aps1,
                            X1[u % 2], X1[(u + 1) % 2], PO1, Y1, TC1,
                            H1[(u + 1) % 2][:].rearrange(
                                "p (k b) -> p k b", k=KC
                            ),
                        )
                    # ---- xw1(iw) = A1 @ h0win + b1 (for next iteration) ----
                    for gc in range(GC):
                        px1 = tps_pool.tile([128, WIN * PB], F32, tag="px")
                        for kc in range(KC):
                            nc.tensor.matmul(
                                px1[:],
                                a1_sb[:, kc, gc * 128:(gc + 1) * 128],
                                hwin[:, kc].rearrange("p w b -> p (w b)"),
                                start=(kc == 0),
                                stop=(kc == KC - 1),
                            )
                        nc.vector.tensor_scalar_add(
                            xw1_sb[:, gc].rearrange("p w b -> p (w b)"),
                            px1[:], b1_sb[:, gc:gc + 1],
                        )

            # ---- epilogue: out = relu(h1_last @ Wout.T + bout) ----
            with tc.tile_pool(name="fps", bufs=2, space="PSUM") as fps_pool, \
                 tc.tile_pool(name="fo", bufs=1) as fo_pool:
                h1_last = H1[0]  # WIN even -> state lands in H1[0]
                out_sb = fo_pool.tile([PB, OUT], F32)
                for half in range(2):
                    psF = fps_pool.tile([PB, 512], F32, tag="psF")
                    for kc in range(KC):
                        nc.tensor.matmul(
                            psF[:],
                            h1_last[:, kc * PB:(kc + 1) * PB],
                            wout_sb[:, kc, half * 512:(half + 1) * 512],
                            start=(kc == 0),
                            stop=(kc == KC - 1),
                        )
                    sl = slice(half * 512, (half + 1) * 512)
                    nc.vector.tensor_tensor(
                        out_sb[:, sl], psF[:], bout_sb[:, sl], OP.add
                    )
                    nc.vector.tensor_scalar_max(out_sb[:, sl], out_sb[:, sl], 0.0)
                nc.sync.dma_start(out_ext[:, :], out_sb[:])

    _split_drain_waits(nc)
    return nc


_NC_CACHE = None


def _get_nc():
    global _NC_CACHE
    if _NC_CACHE is None:
        _NC_CACHE = build_nc()
    return _NC_CACHE


def _prep_host(inputs):
    bf = ml_dtypes.bfloat16
    x = np.asarray(inputs["x"], dtype=np.float32)

    coef = 1.0 / math.gamma(0.5)
    t = np.arange(T, dtype=np.float64)
    diff = t[:, None] - t[None, :]
    W = np.where(diff > 0, (np.abs(diff) + 1e-6) ** -0.5, 0.0).astype(np.float32)
    d = (coef * W.sum(1)).astype(np.float32)
    G = (np.diag(d) - coef * W).astype(np.float32)      # feats_b = G @ x_b
    GTp = np.zeros((T, TPAD), np.float32)               # (t', t) padded
    GTp[:, :T] = G.T
    GT = np.ascontiguousarray(GTp.reshape(KC, 128, TPAD)).astype(bf)

    perm = np.concatenate([  # torch gate order i,f,g,o -> [i,f,o,g]
        np.arange(0, H), np.arange(H, 2 * H),
        np.arange(3 * H, 4 * H), np.arange(2 * H, 3 * H),
    ])

    A0 = np.zeros((G4, NP), np.float32)
    A0[:, :N] = np.asarray(inputs["Wih0"], np.float32)[perm, :N]
    A0T = np.ascontiguousarray(A0.T).astype(bf).reshape(MC, 128, G4)
    b0 = (np.asarray(inputs["bih0"], np.float32)
          + np.asarray(inputs["bhh0"], np.float32))[perm]
    b0_t = np.ascontiguousarray(b0.reshape(GC, 128).T)
    Whh0T = np.ascontiguousarray(
        np.asarray(inputs["Whh0"], np.float32)[perm].T
    ).astype(bf).reshape(KC, 128, G4)

    A1T = np.ascontiguousarray(
        np.asarray(inputs["Wih1"], np.float32)[perm].T
    ).astype(bf).reshape(KC, 128, G4)
    b1 = (np.asarray(inputs["bih1"], np.float32)
          + np.asarray(inputs["bhh1"], np.float32))[perm]
    b1_t = np.ascontiguousarray(b1.reshape(GC, 128).T)
    Whh1T = np.ascontiguousarray(
        np.asarray(inputs["Whh1"], np.float32)[perm].T
    ).astype(bf).reshape(KC, 128, G4)

    WoutT = np.ascontiguousarray(
        np.asarray(inputs["Wout"], np.float32).T
    ).astype(bf).reshape(KC, 128, OUT)
    bout_r = np.broadcast_to(
        np.asarray(inputs["bout"], np.float32), (PB, OUT)
    ).copy()

    # caputo stationary: [b, kc, p, n] = x[b, t'=kc*128+p, n], N padded
    xp = np.zeros((B, T, NP), np.float32)
    xp[:, :, :N] = x
    xt2 = xp.reshape(B, KC, 128, NP).astype(bf)

    shared = dict(
        gt=GT, a0t=A0T, b0=b0_t, whh0t=Whh0T, a1t=A1T, b1=b1_t,
        whh1t=Whh1T, woutt=WoutT, boutr=bout_r,
    )
    in_maps = []
    for c in range(NCORES):
        m = dict(shared)
        m["xt"] = np.ascontiguousarray(xt2[c * PB:(c + 1) * PB])
        in_maps.append(m)
    return in_maps


def kernel(**inputs):
    nc = _get_nc()
    in_maps = _prep_host(inputs)
    res = run_bass_kernel_spmd(nc, in_maps, core_ids=list(range(NCORES)))
    out = np.concatenate([r["out"] for r in res.results], axis=0)
    return out.astype(np.float32)


# revision 3
# speedup vs baseline: 1.3283x; 1.3283x over previous
"""Trainium2 Bass kernel for CaputoEncoder — layer-split pipeline (v3).

Cores 0-3 run LSTM layer 0 for batch groups of 16; cores 4-7 run layer 1 for
the same groups, one segment behind, receiving h0 windows via pairwise
AllGather ({i, i+4}). One uniform SPMD program; the role difference is
expressed purely through per-core input data:

  role L0 (cores 0-3):  gt = G^T (caputo), x = x-batch, a_mine = A0,
                        a_recv = 0, whh = Whh0, bias = b0, v = ones
  role L1 (cores 4-7):  gt = 0, x = 0, a_mine = 0, a_recv = A1,
                        whh = Whh1, bias = b1, v = 0 for the first segment

Every core, per window: caputo matmuls (junk for L1), xw psum =
A_mine@feats + A_recv@recv_slot0 (+ v-scaled bias), a 32-step scan, h window
DMA'd to the send buffer. After each python-level segment, one AllGather
ships the segment's h windows; slot 0 of its output is always the L0 core's
h0, so every core reads slot 0 (L0 cores then multiply it by a_recv = 0).
L1's timeline is shifted one segment; zero xw windows keep its state exactly
zero until real data arrives, and the final linear layer (junk on L0 cores)
is taken from cores 4-7 by the host.
"""

import math
from contextlib import ExitStack

import numpy as np
import ml_dtypes

import concourse.bass as bass
import concourse.tile as tile
from concourse import mybir
from concourse.bass import ds
from concourse.bass_utils import run_bass_kernel_spmd

AF = mybir.ActivationFunctionType
OP = mybir.AluOpType
F32 = mybir.dt.float32
BF16 = mybir.dt.bfloat16

B, T, N = 64, 512, 250
H = 512
G4 = 4 * H
OUT = 1024
NCORES = 8
NPAIR = 4
PB = B // NPAIR           # 16 batches per core pair
WIN = 32
NWIN = T // WIN           # 16 real windows
NSEG = 4                  # python-level segments (CC boundaries)
WPS = NWIN // NSEG        # 4 windows per segment
NSEG_T = NSEG + 1         # +1 segment of L1 lag
TWIN = NSEG_T * WPS       # 20 total window slots
TPAD = TWIN * WIN         # 640 G^T columns
KC = H // 128
GC = G4 // 128
MC = 2
NP = MC * 128
CB = KC * PB              # 64 h/c columns
PAIRS = [[0, 4], [1, 5], [2, 6], [3, 7]]


def _split_drain_waits(nc, max_waits=1):
    for bb in nc.m.functions[0].blocks:
        insts = bb.instructions
        i = 0
        while i < len(insts):
            ins = insts[i]
            si = ins.sync_info
            if si is not None and len(si.on_wait) > max_waits:
                waits = list(si.on_wait)
                ins.sync_info = mybir.SyncInfo(
                    on_wait=waits[:max_waits], on_update=list(si.on_update)
                )
                for j, w in enumerate(waits[max_waits:]):
                    nop = mybir.InstNoOp(name=f"{ins.name}-wsplit{j}")
                    nop.engine = ins.engine
                    nop.sync_info = mybir.SyncInfo(on_wait=[w], on_update=[])
                    insts.insert(i, nop)
                    i += 1
            i += 1


def build_nc():
    nc = bass.Bass()

    x_in = nc.dram_tensor("xt", [PB, KC, 128, NP], BF16, kind="ExternalInput")
    gt_ins = [nc.dram_tensor(f"gt{s}", [KC, 128, WPS * WIN], BF16, kind="ExternalInput")
               for s in range(NSEG_T)]
    am_in = nc.dram_tensor("amt", [MC, 128, G4], BF16, kind="ExternalInput")
    ar_in = nc.dram_tensor("art", [KC, 128, G4], BF16, kind="ExternalInput")
    b_in = nc.dram_tensor("bg", [128, GC], F32, kind="ExternalInput")
    v0_in = nc.dram_tensor("v0", [128, 1], F32, kind="ExternalInput")
    whh_in = nc.dram_tensor("whht", [KC, 128, G4], BF16, kind="ExternalInput")
    wout_in = nc.dram_tensor("woutt", [KC, 128, OUT], BF16, kind="ExternalInput")
    bout_in = nc.dram_tensor("boutr", [PB, OUT], F32, kind="ExternalInput")
    out_ext = nc.dram_tensor("out", [PB, OUT], F32, kind="ExternalOutput")

    send_dram = nc.dram_tensor("sendb", [WPS, 128, KC * WIN * PB], BF16)
    recv_dram = nc.dram_tensor("recvb", [2 * WPS, 128, KC * WIN * PB], BF16)

    with tile.TileContext(nc) as tc:
        with ExitStack() as ctx:
            cpool = ctx.enter_context(tc.tile_pool(name="consts", bufs=1))
            spool = ctx.enter_context(tc.tile_pool(name="state", bufs=1))

            x_sb = cpool.tile([128, PB, KC, NP], BF16)
            nc.sync.dma_start(
                x_sb[:], x_in[:, :, :, :].rearrange("b k p n -> p b k n")
            )
            am_sb = cpool.tile([128, MC, G4], BF16)
            nc.sync.dma_start(am_sb[:], am_in[:, :, :].rearrange("k p g -> p k g"))
            ar_sb = cpool.tile([128, KC, G4], BF16)
            nc.sync.dma_start(ar_sb[:], ar_in[:, :, :].rearrange("k p g -> p k g"))
            b_sb = cpool.tile([128, GC], F32)
            nc.sync.dma_start(b_sb[:], b_in[:, :])
            whh_sb = cpool.tile([128, KC, G4], BF16)
            nc.sync.dma_start(whh_sb[:], whh_in[:, :, :].rearrange("k p g -> p k g"))
            wout_sb = cpool.tile([128, KC, OUT], BF16)
            nc.sync.dma_start(wout_sb[:], wout_in[:, :, :].rearrange("k p g -> p k g"))
            bout_sb = cpool.tile([PB, OUT], F32)
            nc.sync.dma_start(bout_sb[:], bout_in[:, :])
            v0_sb = cpool.tile([128, 1], F32)
            nc.sync.dma_start(v0_sb[:], v0_in[:, :])

            xw_sb = spool.tile([128, GC, WIN, PB], BF16)
            hwin = spool.tile([128, KC, WIN, PB], BF16)
            feats = spool.tile([128, MC, WIN, PB], BF16)
            recv_sb = spool.tile([128, KC, WIN, PB], BF16)
            Hlast = spool.tile([128, CB], BF16)
            X = [spool.tile([128, 2 * CB], F32, name=f"X_{i}") for i in range(2)]
            beff = spool.tile([128, GC], F32)
            zt = spool.tile([128, KC * WIN * PB], BF16)
            nc.vector.memset(Hlast[:], 0.0)
            nc.vector.memset(X[0][:], 0.0)
            nc.vector.memset(X[1][:], 0.0)
            nc.vector.memset(zt[:], 0.0)
            for jw in range(WPS):
                nc.sync.dma_start(recv_dram[jw, :, :], zt[:])

            with tc.tile_pool(name="gtw", bufs=2) as gtw_pool, \
                 tc.tile_pool(name="tps", bufs=2, space="PSUM") as tps_pool, \
                 tc.tile_pool(name="ps", bufs=2, space="PSUM") as ps_pool, \
                 tc.tile_pool(name="ew", bufs=3) as ew_pool:
                for seg in range(NSEG_T):
                    if seg == 0:
                        nc.gpsimd.tensor_scalar_mul(
                            beff[:], b_sb[:], v0_sb[:, 0:1])
                    elif seg == 1:
                        nc.gpsimd.tensor_copy(beff[:], b_sb[:])
                    with tc.For_i(0, WPS, 1,
                                  hint_engines=(mybir.EngineType.PE,)) as jw:
                        # ---- per-window DMAs ----
                        gtw = gtw_pool.tile([128, KC, WIN], BF16, tag="gtw")
                        nc.sync.dma_start(
                            gtw[:],
                            gt_ins[seg][:, :, ds(jw * WIN, WIN)].rearrange(
                                "k p w -> p k w"),
                        )
                        nc.sync.dma_start(
                            recv_sb[:].rearrange("p k w b -> p (k w b)"),
                            recv_dram[ds(jw, 1), :, :].rearrange(
                                "w p c -> p w c"),
                        )
                        # ---- caputo feats ----
                        for b in range(PB):
                            for mc in range(MC):
                                psC = tps_pool.tile([128, WIN], F32, tag="psC")
                                for kc in range(KC):
                                    nc.tensor.matmul(
                                        psC[:],
                                        x_sb[:, b, kc, mc * 128:(mc + 1) * 128],
                                        gtw[:, kc, :],
                                        start=(kc == 0),
                                        stop=(kc == KC - 1),
                                    )
                                nc.scalar.activation(
                                    feats[:, mc, :, b], psC[:], AF.Copy
                                )
                        # ---- xw = A_mine @ feats + A_recv @ recv + v*b ----
                        for gc in range(GC):
                            px = tps_pool.tile([128, WIN * PB], F32, tag="px")
                            for mc in range(MC):
                                nc.tensor.matmul(
                                    px[:],
                                    am_sb[:, mc, gc * 128:(gc + 1) * 128],
                                    feats[:, mc].rearrange("p w b -> p (w b)"),
                                    start=(mc == 0),
                                    stop=False,
                                )
                            for kc in range(KC):
                                nc.tensor.matmul(
                                    px[:],
                                    ar_sb[:, kc, gc * 128:(gc + 1) * 128],
                                    recv_sb[:, kc].rearrange("p w b -> p (w b)"),
                                    start=False,
                                    stop=(kc == KC - 1),
                                )
                            nc.vector.tensor_scalar_add(
                                xw_sb[:, gc].rearrange("p w b -> p (w b)"),
                                px[:], beff[:, gc:gc + 1],
                            )
                        # ---- scan 32 steps ----
                        for u in range(WIN):
                            ps = ps_pool.tile([128, GC * PB], F32, tag="ps")
                            nc.scalar.copy(
                                ps[:].rearrange("p (g b) -> p g b", g=GC),
                                xw_sb[:, :, u, :],
                            )
                            h_aps = (
                                [Hlast[:, kc * PB:(kc + 1) * PB]
                                 for kc in range(KC)]
                                if u == 0 else
                                [hwin[:, kc, u - 1, :] for kc in range(KC)]
                            )
                            for gc in range(GC):
                                for kc in range(KC):
                                    nc.tensor.matmul(
                                        ps[:, gc * PB:(gc + 1) * PB],
                                        whh_sb[:, kc, gc * 128:(gc + 1) * 128],
                                        h_aps[kc],
                                        start=False,
                                        stop=(kc == KC - 1),
                                        skip_group_check=True,
                                    )
                            PO = ew_pool.tile([128, 3 * CB], F32, tag="PO")
                            Y = ew_pool.tile([128, 2 * CB], F32, tag="Y")
                            TC_ = ew_pool.tile([128, CB], F32, tag="TC")
                            X_cur, X_next = X[u % 2], X[(u + 1) % 2]
                            nc.scalar.activation(PO[:], ps[:, :3 * CB], AF.Sigmoid)
                            nc.scalar.activation(
                                X_cur[:, :CB], ps[:, 3 * CB:], AF.Tanh)
                            nc.vector.tensor_tensor(
                                Y[:], PO[:, :2 * CB], X_cur[:], OP.mult)
                            nc.vector.tensor_tensor(
                                X_next[:, CB:], Y[:, :CB], Y[:, CB:], OP.add)
                            nc.scalar.activation(
                                TC_[:], X_next[:, CB:], AF.Tanh)
                            nc.vector.tensor_tensor(
                                hwin[:, :, u, :], PO[:, 2 * CB:], TC_[:],
                                OP.mult)
                            if u == WIN - 1:
                                nc.gpsimd.tensor_copy(
                                    Hlast[:].rearrange("p (k b) -> p k b", k=KC),
                                    hwin[:, :, u, :],
                                )
                        # ---- ship h window ----
                        nc.sync.dma_start(
                            send_dram[ds(jw, 1), :, :],
                            hwin[:].rearrange("p k w b -> p (k w b)"),
                        )
                    if seg < NSEG:
                        nc.gpsimd.collective_compute(
                            "AllGather",
                            mybir.AluOpType.bypass,
                            PAIRS,
                            ins=[send_dram[:, :, :]],
                            outs=[recv_dram[:, :, :]],
                        )

            # ---- epilogue: out = relu(h_last @ Wout.T + bout) ----
            with tc.tile_pool(name="fps", bufs=2, space="PSUM") as fps_pool, \
                 tc.tile_pool(name="fo", bufs=1) as fo_pool:
                out_sb = fo_pool.tile([PB, OUT], F32)
                for half in range(2):
                    psF = fps_pool.tile([PB, 512], F32, tag="psF")
                    for kc in range(KC):
                        nc.tensor.matmul(
                            psF[:],
                            Hlast[:, kc * PB:(kc + 1) * PB],
                            wout_sb[:, kc, half * 512:(half + 1) * 512],
                            start=(kc == 0),
                            stop=(kc == KC - 1),
                        )
                    sl = slice(half * 512, (half + 1) * 512)
                    nc.vector.tensor_tensor(
                        out_sb[:, sl], psF[:], bout_sb[:, sl], OP.add
                    )
                    nc.vector.tensor_scalar_max(out_sb[:, sl], out_sb[:, sl], 0.0)
                nc.sync.dma_start(out_ext[:, :], out_sb[:])

    _split_drain_waits(nc)
    return nc


_NC_CACHE = None


def _get_nc():
    global _NC_CACHE
    if _NC_CACHE is None:
        _NC_CACHE = build_nc()
    return _NC_CACHE


def _prep_host(inputs):
    bf = ml_dtypes.bfloat16
    x = np.asarray(inputs["x"], dtype=np.float32)

    coef = 1.0 / math.gamma(0.5)
    t = np.arange(T, dtype=np.float64)
    diff = t[:, None] - t[None, :]
    W = np.where(diff > 0, (np.abs(diff) + 1e-6) ** -0.5, 0.0).astype(np.float32)
    d = (coef * W.sum(1)).astype(np.float32)
    G = (np.diag(d) - coef * W).astype(np.float32)
    GTp = np.zeros((T, TPAD), np.float32)
    GTp[:, :T] = G.T
    GT = np.ascontiguousarray(
        GTp.reshape(KC, 128, NSEG_T, WPS * WIN).transpose(2, 0, 1, 3)
    ).astype(bf)
    GT0 = np.zeros_like(GT)

    perm = np.concatenate([
        np.arange(0, H), np.arange(H, 2 * H),
        np.arange(3 * H, 4 * H), np.arange(2 * H, 3 * H),
    ])

    A0 = np.zeros((G4, NP), np.float32)
    A0[:, :N] = np.asarray(inputs["Wih0"], np.float32)[perm, :N]
    A0T = np.ascontiguousarray(A0.T).astype(bf).reshape(MC, 128, G4)
    b0 = (np.asarray(inputs["bih0"], np.float32)
          + np.asarray(inputs["bhh0"], np.float32))[perm]
    b0_t = np.ascontiguousarray(b0.reshape(GC, 128).T)
    Whh0T = np.ascontiguousarray(
        np.asarray(inputs["Whh0"], np.float32)[perm].T
    ).astype(bf).reshape(KC, 128, G4)

    A1T = np.ascontiguousarray(
        np.asarray(inputs["Wih1"], np.float32)[perm].T
    ).astype(bf).reshape(KC, 128, G4)
    b1 = (np.asarray(inputs["bih1"], np.float32)
          + np.asarray(inputs["bhh1"], np.float32))[perm]
    b1_t = np.ascontiguousarray(b1.reshape(GC, 128).T)
    Whh1T = np.ascontiguousarray(
        np.asarray(inputs["Whh1"], np.float32)[perm].T
    ).astype(bf).reshape(KC, 128, G4)

    WoutT = np.ascontiguousarray(
        np.asarray(inputs["Wout"], np.float32).T
    ).astype(bf).reshape(KC, 128, OUT)
    bout_r = np.broadcast_to(
        np.asarray(inputs["bout"], np.float32), (PB, OUT)
    ).copy()

    xp = np.zeros((B, T, NP), np.float32)
    xp[:, :, :N] = x
    xt2 = xp.reshape(B, KC, 128, NP).astype(bf)
    xz = np.zeros((PB, KC, 128, NP), bf)

    zero_mc = np.zeros((MC, 128, G4), bf)
    zero_kc = np.zeros((KC, 128, G4), bf)
    v0_l0 = np.ones((128, 1), np.float32)
    v0_l1 = np.zeros((128, 1), np.float32)

    in_maps = []
    for c in range(NCORES):
        is_l1 = c >= NPAIR
        g = c % NPAIR
        gts = GT0 if is_l1 else GT
        m = dict(
            woutt=WoutT, boutr=bout_r,
            xt=xz if is_l1 else np.ascontiguousarray(xt2[g * PB:(g + 1) * PB]),
            amt=zero_mc if is_l1 else A0T,
            art=A1T if is_l1 else zero_kc,
            bg=b1_t if is_l1 else b0_t,
            whht=Whh1T if is_l1 else Whh0T,
            v0=v0_l1 if is_l1 else v0_l0,
        )
        for s in range(NSEG_T):
            m[f"gt{s}"] = np.ascontiguousarray(gts[s])
        in_maps.append(m)
    return in_maps


def kernel(**inputs):
    nc = _get_nc()
    in_maps = _prep_host(inputs)
    res = run_bass_kernel_spmd(nc, in_maps, core_ids=list(range(NCORES)))
    out = np.concatenate(
        [res.results[NPAIR + g]["out"] for g in range(NPAIR)], axis=0
    )
    return out.astype(np.float32)


# revision 4
# speedup vs baseline: 1.3831x; 1.0412x over previous
"""Trainium2 Bass kernel for CaputoEncoder — layer-split pipeline (v3).

Cores 0-3 run LSTM layer 0 for batch groups of 16; cores 4-7 run layer 1 for
the same groups, one segment behind, receiving h0 windows via pairwise
AllGather ({i, i+4}). One uniform SPMD program; the role difference is
expressed purely through per-core input data:

  role L0 (cores 0-3):  gt = G^T (caputo), x = x-batch, a_mine = A0,
                        a_recv = 0, whh = Whh0, bias = b0, v = ones
  role L1 (cores 4-7):  gt = 0, x = 0, a_mine = 0, a_recv = A1,
                        whh = Whh1, bias = b1, v = 0 for the first segment

Every core, per window: caputo matmuls (junk for L1), xw psum =
A_mine@feats + A_recv@recv_slot0 (+ v-scaled bias), a 32-step scan, h window
DMA'd to the send buffer. After each python-level segment, one AllGather
ships the segment's h windows; slot 0 of its output is always the L0 core's
h0, so every core reads slot 0 (L0 cores then multiply it by a_recv = 0).
L1's timeline is shifted one segment; zero xw windows keep its state exactly
zero until real data arrives, and the final linear layer (junk on L0 cores)
is taken from cores 4-7 by the host.
"""

import math
from contextlib import ExitStack

import numpy as np
import ml_dtypes

import concourse.bass as bass
import concourse.tile as tile
from concourse import mybir
from concourse.bass import ds
from concourse.bass_utils import run_bass_kernel_spmd

AF = mybir.ActivationFunctionType
OP = mybir.AluOpType
F32 = mybir.dt.float32
BF16 = mybir.dt.bfloat16

B, T, N = 64, 512, 250
H = 512
G4 = 4 * H
OUT = 1024
NCORES = 8
NPAIR = 4
PB = B // NPAIR           # 16 batches per core pair
WIN = 32
NWIN = T // WIN           # 16 real windows
NSEG = 4                  # python-level segments (CC boundaries)
WPS = NWIN // NSEG        # 4 windows per segment
NSEG_T = NSEG + 1         # +1 segment of L1 lag
TWIN = NSEG_T * WPS       # 20 total window slots
TPAD = TWIN * WIN         # 640 G^T columns
KC = H // 128
GC = G4 // 128
MC = 2
NP = MC * 128
CB = KC * PB              # 64 h/c columns
PAIRS = [[0, 4], [1, 5], [2, 6], [3, 7]]


def _split_drain_waits(nc, max_waits=1):
    for bb in nc.m.functions[0].blocks:
        insts = bb.instructions
        i = 0
        while i < len(insts):
            ins = insts[i]
            si = ins.sync_info
            if si is not None and len(si.on_wait) > max_waits:
                waits = list(si.on_wait)
                ins.sync_info = mybir.SyncInfo(
                    on_wait=waits[:max_waits], on_update=list(si.on_update)
                )
                for j, w in enumerate(waits[max_waits:]):
                    nop = mybir.InstNoOp(name=f"{ins.name}-wsplit{j}")
                    nop.engine = ins.engine
                    nop.sync_info = mybir.SyncInfo(on_wait=[w], on_update=[])
                    insts.insert(i, nop)
                    i += 1
            i += 1


def build_nc():
    nc = bass.Bass()

    x_in = nc.dram_tensor("xt", [PB, KC, 128, NP], BF16, kind="ExternalInput")
    gt_ins = [nc.dram_tensor(f"gt{s}", [KC, 128, WPS * WIN], BF16, kind="ExternalInput")
               for s in range(NSEG_T)]
    am_in = nc.dram_tensor("amt", [MC, 128, G4], BF16, kind="ExternalInput")
    ar_in = nc.dram_tensor("art", [KC, 128, G4], BF16, kind="ExternalInput")
    b_in = nc.dram_tensor("bg", [128, GC], F32, kind="ExternalInput")
    v0_in = nc.dram_tensor("v0", [128, 1], F32, kind="ExternalInput")
    whh_in = nc.dram_tensor("whht", [KC, 128, G4], BF16, kind="ExternalInput")
    wout_in = nc.dram_tensor("woutt", [KC, 128, OUT], BF16, kind="ExternalInput")
    bout_in = nc.dram_tensor("boutr", [PB, OUT], F32, kind="ExternalInput")
    out_ext = nc.dram_tensor("out", [PB, OUT], F32, kind="ExternalOutput")

    send_dram = nc.dram_tensor("sendb", [WPS, 128, KC * WIN * PB], BF16)
    recv_dram = nc.dram_tensor("recvb", [2 * WPS, 128, KC * WIN * PB], BF16)

    with tile.TileContext(nc) as tc:
        with ExitStack() as ctx:
            cpool = ctx.enter_context(tc.tile_pool(name="consts", bufs=1))
            spool = ctx.enter_context(tc.tile_pool(name="state", bufs=1))

            x_sb = cpool.tile([128, PB, KC, NP], BF16)
            nc.sync.dma_start(
                x_sb[:], x_in[:, :, :, :].rearrange("b k p n -> p b k n")
            )
            am_sb = cpool.tile([128, MC, G4], BF16)
            nc.sync.dma_start(am_sb[:], am_in[:, :, :].rearrange("k p g -> p k g"))
            ar_sb = cpool.tile([128, KC, G4], BF16)
            nc.sync.dma_start(ar_sb[:], ar_in[:, :, :].rearrange("k p g -> p k g"))
            b_sb = cpool.tile([128, GC], F32)
            nc.sync.dma_start(b_sb[:], b_in[:, :])
            whh_sb = cpool.tile([128, KC, G4], BF16)
            nc.sync.dma_start(whh_sb[:], whh_in[:, :, :].rearrange("k p g -> p k g"))
            wout_sb = cpool.tile([128, KC, OUT], BF16)
            nc.sync.dma_start(wout_sb[:], wout_in[:, :, :].rearrange("k p g -> p k g"))
            bout_sb = cpool.tile([PB, OUT], F32)
            nc.sync.dma_start(bout_sb[:], bout_in[:, :])
            v0_sb = cpool.tile([128, 1], F32)
            nc.sync.dma_start(v0_sb[:], v0_in[:, :])

            xw_sb = spool.tile([128, GC, WIN, PB], BF16)
            hwin = spool.tile([128, KC, WIN, PB], BF16)
            feats = spool.tile([128, MC, WIN, PB], BF16)
            recv_sb = spool.tile([128, KC, WIN, PB], BF16)
            Hlast = spool.tile([128, CB], BF16)
            X = [spool.tile([128, 2 * CB], F32, name=f"X_{i}") for i in range(2)]
            beff = spool.tile([128, GC], F32)
            zt = spool.tile([128, KC * WIN * PB], BF16)
            nc.vector.memset(Hlast[:], 0.0)
            nc.vector.memset(X[0][:], 0.0)
            nc.vector.memset(X[1][:], 0.0)
            nc.vector.memset(zt[:], 0.0)
            for jw in range(WPS):
                nc.sync.dma_start(recv_dram[jw, :, :], zt[:])

            with tc.tile_pool(name="gtw", bufs=2) as gtw_pool, \
                 tc.tile_pool(name="tps", bufs=2, space="PSUM") as tps_pool, \
                 tc.tile_pool(name="ps", bufs=2, space="PSUM") as ps_pool, \
                 tc.tile_pool(name="ew", bufs=3) as ew_pool:
                for seg in range(NSEG_T):
                    if seg == 0:
                        nc.gpsimd.tensor_scalar_mul(
                            beff[:], b_sb[:], v0_sb[:, 0:1])
                    elif seg == 1:
                        nc.gpsimd.tensor_copy(beff[:], b_sb[:])
                    with tc.For_i(0, WPS, 1,
                                  hint_engines=(mybir.EngineType.PE,)) as jw:
                        # ---- per-window DMAs ----
                        gtw = gtw_pool.tile([128, KC, WIN], BF16, tag="gtw")
                        nc.sync.dma_start(
                            gtw[:],
                            gt_ins[seg][:, :, ds(jw * WIN, WIN)].rearrange(
                                "k p w -> p k w"),
                        )
                        nc.sync.dma_start(
                            recv_sb[:].rearrange("p k w b -> p (k w b)"),
                            recv_dram[ds(jw, 1), :, :].rearrange(
                                "w p c -> p w c"),
                        )
                        # ---- caputo feats ----
                        for b in range(PB):
                            for mc in range(MC):
                                psC = tps_pool.tile([128, WIN], F32, tag="psC")
                                for kc in range(KC):
                                    nc.tensor.matmul(
                                        psC[:],
                                        x_sb[:, b, kc, mc * 128:(mc + 1) * 128],
                                        gtw[:, kc, :],
                                        start=(kc == 0),
                                        stop=(kc == KC - 1),
                                    )
                                nc.scalar.activation(
                                    feats[:, mc, :, b], psC[:], AF.Copy
                                )
                        # ---- xw = A_mine @ feats + A_recv @ recv + v*b ----
                        for gc in range(GC):
                            px = tps_pool.tile([128, WIN * PB], F32, tag="px")
                            for mc in range(MC):
                                nc.tensor.matmul(
                                    px[:],
                                    am_sb[:, mc, gc * 128:(gc + 1) * 128],
                                    feats[:, mc].rearrange("p w b -> p (w b)"),
                                    start=(mc == 0),
                                    stop=False,
                                )
                            for kc in range(KC):
                                nc.tensor.matmul(
                                    px[:],
                                    ar_sb[:, kc, gc * 128:(gc + 1) * 128],
                                    recv_sb[:, kc].rearrange("p w b -> p (w b)"),
                                    start=False,
                                    stop=(kc == KC - 1),
                                )
                            nc.vector.tensor_scalar_add(
                                xw_sb[:, gc].rearrange("p w b -> p (w b)"),
                                px[:], beff[:, gc:gc + 1],
                            )
                        # ---- scan 32 steps ([g,i,f,o], split psums) ----
                        for u in range(WIN):
                            pgo = ps_pool.tile([128, 2 * CB], F32, tag="pgo")
                            psif = ps_pool.tile([128, 2 * CB], F32, tag="psif")
                            psg = pgo[:, :CB]
                            pso = pgo[:, CB:]
                            nc.scalar.copy(
                                psg.rearrange("p (g b) -> p g b", g=KC),
                                xw_sb[:, 0:4, u, :])
                            nc.scalar.copy(
                                psif[:].rearrange("p (g b) -> p g b", g=2 * KC),
                                xw_sb[:, 4:12, u, :])
                            nc.scalar.copy(
                                pso.rearrange("p (g b) -> p g b", g=KC),
                                xw_sb[:, 12:16, u, :])
                            h_aps = (
                                [Hlast[:, kc * PB:(kc + 1) * PB]
                                 for kc in range(KC)]
                                if u == 0 else
                                [hwin[:, kc, u - 1, :] for kc in range(KC)]
                            )

                            def quad(ps_t, g0, g1):
                                for gc in range(g0, g1):
                                    for kc in range(KC):
                                        nc.tensor.matmul(
                                            ps_t[:, (gc - g0) * PB:
                                                 (gc - g0 + 1) * PB],
                                            whh_sb[:, kc,
                                                   gc * 128:(gc + 1) * 128],
                                            h_aps[kc],
                                            start=False,
                                            stop=(kc == KC - 1),
                                            skip_group_check=True,
                                        )


                            PO = ew_pool.tile([128, 2 * CB], F32, tag="PO")
                            O_ = ew_pool.tile([128, CB], F32, tag="O")
                            Y = ew_pool.tile([128, 2 * CB], F32, tag="Y")
                            TC_ = ew_pool.tile([128, CB], F32, tag="TC")
                            X_cur, X_next = X[u % 2], X[(u + 1) % 2]
                            quad(psg, 0, 4)
                            nc.scalar.activation(
                                X_cur[:, :CB], psg, AF.Tanh)
                            quad(psif, 4, 12)
                            nc.scalar.activation(PO[:], psif[:], AF.Sigmoid)
                            nc.vector.tensor_tensor(
                                Y[:], PO[:], X_cur[:], OP.mult)
                            nc.vector.tensor_tensor(
                                X_next[:, CB:], Y[:, :CB], Y[:, CB:], OP.add)
                            nc.scalar.activation(
                                TC_[:], X_next[:, CB:], AF.Tanh)
                            quad(pso, 12, 16)
                            nc.scalar.activation(O_[:], pso, AF.Sigmoid)
                            nc.vector.tensor_tensor(
                                hwin[:, :, u, :], O_[:], TC_[:], OP.mult)
                            if u == WIN - 1:
                                nc.gpsimd.tensor_copy(
                                    Hlast[:].rearrange("p (k b) -> p k b", k=KC),
                                    hwin[:, :, u, :],
                                )
                        # ---- ship h window ----
                        nc.sync.dma_start(
                            send_dram[ds(jw, 1), :, :],
                            hwin[:].rearrange("p k w b -> p (k w b)"),
                        )
                    if seg < NSEG:
                        nc.gpsimd.collective_compute(
                            "AllGather",
                            mybir.AluOpType.bypass,
                            PAIRS,
                            ins=[send_dram[:, :, :]],
                            outs=[recv_dram[:, :, :]],
                        )

            # ---- epilogue: out = relu(h_last @ Wout.T + bout) ----
            with tc.tile_pool(name="fps", bufs=2, space="PSUM") as fps_pool, \
                 tc.tile_pool(name="fo", bufs=1) as fo_pool:
                out_sb = fo_pool.tile([PB, OUT], F32)
                for half in range(2):
                    psF = fps_pool.tile([PB, 512], F32, tag="psF")
                    for kc in range(KC):
                        nc.tensor.matmul(
                            psF[:],
                            Hlast[:, kc * PB:(kc + 1) * PB],
                            wout_sb[:, kc, half * 512:(half + 1) * 512],
                            start=(kc == 0),
                            stop=(kc == KC - 1),
                        )
                    sl = slice(half * 512, (half + 1) * 512)
                    nc.vector.tensor_tensor(
                        out_sb[:, sl], psF[:], bout_sb[:, sl], OP.add
                    )
                    nc.vector.tensor_scalar_max(out_sb[:, sl], out_sb[:, sl], 0.0)
                nc.sync.dma_start(out_ext[:, :], out_sb[:])

    _split_drain_waits(nc)
    return nc


_NC_CACHE = None


def _get_nc():
    global _NC_CACHE
    if _NC_CACHE is None:
        _NC_CACHE = build_nc()
    return _NC_CACHE


def _prep_host(inputs):
    bf = ml_dtypes.bfloat16
    x = np.asarray(inputs["x"], dtype=np.float32)

    coef = 1.0 / math.gamma(0.5)
    t = np.arange(T, dtype=np.float64)
    diff = t[:, None] - t[None, :]
    W = np.where(diff > 0, (np.abs(diff) + 1e-6) ** -0.5, 0.0).astype(np.float32)
    d = (coef * W.sum(1)).astype(np.float32)
    G = (np.diag(d) - coef * W).astype(np.float32)
    GTp = np.zeros((T, TPAD), np.float32)
    GTp[:, :T] = G.T
    GT = np.ascontiguousarray(
        GTp.reshape(KC, 128, NSEG_T, WPS * WIN).transpose(2, 0, 1, 3)
    ).astype(bf)
    GT0 = np.zeros_like(GT)

    perm = np.concatenate([  # torch order i,f,g,o -> [g,i,f,o]
        np.arange(2 * H, 3 * H), np.arange(0, H),
        np.arange(H, 2 * H), np.arange(3 * H, 4 * H),
    ])

    A0 = np.zeros((G4, NP), np.float32)
    A0[:, :N] = np.asarray(inputs["Wih0"], np.float32)[perm, :N]
    A0T = np.ascontiguousarray(A0.T).astype(bf).reshape(MC, 128, G4)
    b0 = (np.asarray(inputs["bih0"], np.float32)
          + np.asarray(inputs["bhh0"], np.float32))[perm]
    b0_t = np.ascontiguousarray(b0.reshape(GC, 128).T)
    Whh0T = np.ascontiguousarray(
        np.asarray(inputs["Whh0"], np.float32)[perm].T
    ).astype(bf).reshape(KC, 128, G4)

    A1T = np.ascontiguousarray(
        np.asarray(inputs["Wih1"], np.float32)[perm].T
    ).astype(bf).reshape(KC, 128, G4)
    b1 = (np.asarray(inputs["bih1"], np.float32)
          + np.asarray(inputs["bhh1"], np.float32))[perm]
    b1_t = np.ascontiguousarray(b1.reshape(GC, 128).T)
    Whh1T = np.ascontiguousarray(
        np.asarray(inputs["Whh1"], np.float32)[perm].T
    ).astype(bf).reshape(KC, 128, G4)

    WoutT = np.ascontiguousarray(
        np.asarray(inputs["Wout"], np.float32).T
    ).astype(bf).reshape(KC, 128, OUT)
    bout_r = np.broadcast_to(
        np.asarray(inputs["bout"], np.float32), (PB, OUT)
    ).copy()

    xp = np.zeros((B, T, NP), np.float32)
    xp[:, :, :N] = x
    xt2 = xp.reshape(B, KC, 128, NP).astype(bf)
    xz = np.zeros((PB, KC, 128, NP), bf)

    zero_mc = np.zeros((MC, 128, G4), bf)
    zero_kc = np.zeros((KC, 128, G4), bf)
    v0_l0 = np.ones((128, 1), np.float32)
    v0_l1 = np.zeros((128, 1), np.float32)

    in_maps = []
    for c in range(NCORES):
        is_l1 = c >= NPAIR
        g = c % NPAIR
        gts = GT0 if is_l1 else GT
        m = dict(
            woutt=WoutT, boutr=bout_r,
            xt=xz if is_l1 else np.ascontiguousarray(xt2[g * PB:(g + 1) * PB]),
            amt=zero_mc if is_l1 else A0T,
            art=A1T if is_l1 else zero_kc,
            bg=b1_t if is_l1 else b0_t,
            whht=Whh1T if is_l1 else Whh0T,
            v0=v0_l1 if is_l1 else v0_l0,
        )
        for s in range(NSEG_T):
            m[f"gt{s}"] = np.ascontiguousarray(gts[s])
        in_maps.append(m)
    return in_maps


def kernel(**inputs):
    nc = _get_nc()
    in_maps = _prep_host(inputs)
    res = run_bass_kernel_spmd(nc, in_maps, core_ids=list(range(NCORES)))
    out = np.concatenate(
        [res.results[NPAIR + g]["out"] for g in range(NPAIR)], axis=0
    )
    return out.astype(np.float32)


# revision 5
# speedup vs baseline: 1.3951x; 1.0087x over previous
"""Trainium2 Bass kernel for CaputoEncoder — layer-split pipeline (v3).

Cores 0-3 run LSTM layer 0 for batch groups of 16; cores 4-7 run layer 1 for
the same groups, one segment behind, receiving h0 windows via pairwise
AllGather ({i, i+4}). One uniform SPMD program; the role difference is
expressed purely through per-core input data:

  role L0 (cores 0-3):  gt = G^T (caputo), x = x-batch, a_mine = A0,
                        a_recv = 0, whh = Whh0, bias = b0, v = ones
  role L1 (cores 4-7):  gt = 0, x = 0, a_mine = 0, a_recv = A1,
                        whh = Whh1, bias = b1, v = 0 for the first segment

Every core, per window: caputo matmuls (junk for L1), xw psum =
A_mine@feats + A_recv@recv_slot0 (+ v-scaled bias), a 32-step scan, h window
DMA'd to the send buffer. After each python-level segment, one AllGather
ships the segment's h windows; slot 0 of its output is always the L0 core's
h0, so every core reads slot 0 (L0 cores then multiply it by a_recv = 0).
L1's timeline is shifted one segment; zero xw windows keep its state exactly
zero until real data arrives, and the final linear layer (junk on L0 cores)
is taken from cores 4-7 by the host.
"""

import math
from contextlib import ExitStack

import numpy as np
import ml_dtypes

import concourse.bass as bass
import concourse.tile as tile
from concourse import mybir
from concourse.bass import ds
from concourse.bass_utils import run_bass_kernel_spmd

AF = mybir.ActivationFunctionType
OP = mybir.AluOpType
F32 = mybir.dt.float32
BF16 = mybir.dt.bfloat16

B, T, N = 64, 512, 250
H = 512
G4 = 4 * H
OUT = 1024
NCORES = 8
NPAIR = 4
PB = B // NPAIR           # 16 batches per core pair
WIN = 32
NWIN = T // WIN           # 16 real windows
NSEG = 4                  # python-level segments (CC boundaries)
WPS = NWIN // NSEG        # 4 windows per segment
NSEG_T = NSEG + 1         # +1 segment of L1 lag
TWIN = NSEG_T * WPS       # 20 total window slots
TPAD = TWIN * WIN         # 640 G^T columns
KC = H // 128
GC = G4 // 128
MC = 2
NP = MC * 128
CB = KC * PB              # 64 h/c columns
PAIRS = [[0, 4], [1, 5], [2, 6], [3, 7]]


def _split_drain_waits(nc, max_waits=1):
    for bb in nc.m.functions[0].blocks:
        insts = bb.instructions
        i = 0
        while i < len(insts):
            ins = insts[i]
            si = ins.sync_info
            if si is not None and len(si.on_wait) > max_waits:
                waits = list(si.on_wait)
                ins.sync_info = mybir.SyncInfo(
                    on_wait=waits[:max_waits], on_update=list(si.on_update)
                )
                for j, w in enumerate(waits[max_waits:]):
                    nop = mybir.InstNoOp(name=f"{ins.name}-wsplit{j}")
                    nop.engine = ins.engine
                    nop.sync_info = mybir.SyncInfo(on_wait=[w], on_update=[])
                    insts.insert(i, nop)
                    i += 1
            i += 1


def build_nc():
    nc = bass.Bass()

    x_in = nc.dram_tensor("xt", [PB, KC, 128, NP], BF16, kind="ExternalInput")
    gt_ins = [nc.dram_tensor(f"gt{s}", [KC, 128, WPS * WIN], BF16, kind="ExternalInput")
               for s in range(NSEG_T)]
    am_in = nc.dram_tensor("amt", [MC, 128, G4], BF16, kind="ExternalInput")
    ar_in = nc.dram_tensor("art", [KC, 128, G4], BF16, kind="ExternalInput")
    b_in = nc.dram_tensor("bg", [128, GC], F32, kind="ExternalInput")
    v0_in = nc.dram_tensor("v0", [128, 1], F32, kind="ExternalInput")
    whh_in = nc.dram_tensor("whht", [KC, 128, G4], BF16, kind="ExternalInput")
    wout_in = nc.dram_tensor("woutt", [KC, 128, OUT], BF16, kind="ExternalInput")
    bout_in = nc.dram_tensor("boutr", [PB, OUT], F32, kind="ExternalInput")
    out_ext = nc.dram_tensor("out", [PB, OUT], F32, kind="ExternalOutput")

    send_dram = nc.dram_tensor("sendb", [WPS, 128, KC * WIN * PB], BF16)
    recv_dram = nc.dram_tensor("recvb", [2 * WPS, 128, KC * WIN * PB], BF16)

    with tile.TileContext(nc) as tc:
        with ExitStack() as ctx:
            cpool = ctx.enter_context(tc.tile_pool(name="consts", bufs=1))
            spool = ctx.enter_context(tc.tile_pool(name="state", bufs=1))

            x_sb = cpool.tile([128, PB, KC, NP], BF16)
            nc.sync.dma_start(
                x_sb[:], x_in[:, :, :, :].rearrange("b k p n -> p b k n")
            )
            am_sb = cpool.tile([128, MC, G4], BF16)
            nc.sync.dma_start(am_sb[:], am_in[:, :, :].rearrange("k p g -> p k g"))
            ar_sb = cpool.tile([128, KC, G4], BF16)
            nc.sync.dma_start(ar_sb[:], ar_in[:, :, :].rearrange("k p g -> p k g"))
            b_sb = cpool.tile([128, GC], F32)
            nc.sync.dma_start(b_sb[:], b_in[:, :])
            whh_sb = cpool.tile([128, KC, G4], BF16)
            nc.sync.dma_start(whh_sb[:], whh_in[:, :, :].rearrange("k p g -> p k g"))
            wout_sb = cpool.tile([128, KC, OUT], BF16)
            nc.sync.dma_start(wout_sb[:], wout_in[:, :, :].rearrange("k p g -> p k g"))
            bout_sb = cpool.tile([PB, OUT], F32)
            nc.sync.dma_start(bout_sb[:], bout_in[:, :])
            v0_sb = cpool.tile([128, 1], F32)
            nc.sync.dma_start(v0_sb[:], v0_in[:, :])

            xw_sb = spool.tile([128, GC, WIN, PB], BF16)
            hwin = spool.tile([128, KC, WIN, PB], BF16)
            feats = spool.tile([128, MC, WIN, PB], BF16)
            recv_sb = spool.tile([128, KC, WIN, PB], BF16)
            Hlast = spool.tile([128, CB], BF16)
            X = [spool.tile([128, 2 * CB], F32, name=f"X_{i}") for i in range(2)]
            beff = spool.tile([128, GC], F32)
            zt = spool.tile([128, KC * WIN * PB], BF16)
            nc.vector.memset(Hlast[:], 0.0)
            nc.vector.memset(X[0][:], 0.0)
            nc.vector.memset(X[1][:], 0.0)
            nc.vector.memset(zt[:], 0.0)
            for jw in range(WPS):
                nc.sync.dma_start(recv_dram[jw, :, :], zt[:])

            with tc.tile_pool(name="gtw", bufs=2) as gtw_pool, \
                 tc.tile_pool(name="tps", bufs=2, space="PSUM") as tps_pool, \
                 tc.tile_pool(name="ps", bufs=2, space="PSUM") as ps_pool, \
                 tc.tile_pool(name="ew", bufs=3) as ew_pool:
                for seg in range(NSEG_T):
                    if seg == 0:
                        nc.gpsimd.tensor_scalar_mul(
                            beff[:], b_sb[:], v0_sb[:, 0:1])
                    elif seg == 1:
                        nc.gpsimd.tensor_copy(beff[:], b_sb[:])
                    with tc.For_i(0, WPS, 1,
                                  hint_engines=(mybir.EngineType.PE,)) as jw:
                        # ---- per-window DMAs ----
                        gtw = gtw_pool.tile([128, KC, WIN], BF16, tag="gtw")
                        nc.sync.dma_start(
                            gtw[:],
                            gt_ins[seg][:, :, ds(jw * WIN, WIN)].rearrange(
                                "k p w -> p k w"),
                        )
                        nc.sync.dma_start(
                            recv_sb[:].rearrange("p k w b -> p (k w b)"),
                            recv_dram[ds(jw, 1), :, :].rearrange(
                                "w p c -> p w c"),
                        )
                        # ---- caputo feats ----
                        for b in range(PB):
                            for mc in range(MC):
                                psC = tps_pool.tile([128, WIN], F32, tag="psC")
                                for kc in range(KC):
                                    nc.tensor.matmul(
                                        psC[:],
                                        x_sb[:, b, kc, mc * 128:(mc + 1) * 128],
                                        gtw[:, kc, :],
                                        start=(kc == 0),
                                        stop=(kc == KC - 1),
                                    )
                                nc.scalar.activation(
                                    feats[:, mc, :, b], psC[:], AF.Copy
                                )
                        # ---- xw = A_mine @ feats + A_recv @ recv + v*b ----
                        for gc in range(GC):
                            px = tps_pool.tile([128, WIN * PB], F32, tag="px")
                            for mc in range(MC):
                                nc.tensor.matmul(
                                    px[:],
                                    am_sb[:, mc, gc * 128:(gc + 1) * 128],
                                    feats[:, mc].rearrange("p w b -> p (w b)"),
                                    start=(mc == 0),
                                    stop=False,
                                )
                            for kc in range(KC):
                                nc.tensor.matmul(
                                    px[:],
                                    ar_sb[:, kc, gc * 128:(gc + 1) * 128],
                                    recv_sb[:, kc].rearrange("p w b -> p (w b)"),
                                    start=False,
                                    stop=(kc == KC - 1),
                                )
                            nc.vector.tensor_scalar_add(
                                xw_sb[:, gc].rearrange("p w b -> p (w b)"),
                                px[:], beff[:, gc:gc + 1],
                            )
                        # ---- scan 32 steps ([g,i,f,o], split psums) ----
                        for u in range(WIN):
                            pgo = ps_pool.tile([128, 2 * CB], F32, tag="pgo")
                            psif = ps_pool.tile([128, 2 * CB], F32, tag="psif")
                            psg = pgo[:, :CB]
                            pso = pgo[:, CB:]
                            nc.vector.tensor_copy(
                                psg.rearrange("p (g b) -> p g b", g=KC),
                                xw_sb[:, 0:4, u, :])
                            nc.vector.tensor_copy(
                                psif[:].rearrange("p (g b) -> p g b", g=2 * KC),
                                xw_sb[:, 4:12, u, :])
                            nc.vector.tensor_copy(
                                pso.rearrange("p (g b) -> p g b", g=KC),
                                xw_sb[:, 12:16, u, :])
                            h_aps = (
                                [Hlast[:, kc * PB:(kc + 1) * PB]
                                 for kc in range(KC)]
                                if u == 0 else
                                [hwin[:, kc, u - 1, :] for kc in range(KC)]
                            )

                            def quad(ps_t, g0, g1):
                                for gc in range(g0, g1):
                                    for kc in range(KC):
                                        nc.tensor.matmul(
                                            ps_t[:, (gc - g0) * PB:
                                                 (gc - g0 + 1) * PB],
                                            whh_sb[:, kc,
                                                   gc * 128:(gc + 1) * 128],
                                            h_aps[kc],
                                            start=False,
                                            stop=(kc == KC - 1),
                                            skip_group_check=True,
                                        )


                            PO = ew_pool.tile([128, 2 * CB], F32, tag="PO")
                            O_ = ew_pool.tile([128, CB], F32, tag="O")
                            Y = ew_pool.tile([128, 2 * CB], F32, tag="Y")
                            TC_ = ew_pool.tile([128, CB], F32, tag="TC")
                            X_cur, X_next = X[u % 2], X[(u + 1) % 2]
                            quad(psg, 0, 4)
                            nc.scalar.activation(
                                X_cur[:, :CB], psg, AF.Tanh)
                            quad(psif, 4, 12)
                            nc.scalar.activation(PO[:], psif[:], AF.Sigmoid)
                            nc.vector.tensor_tensor(
                                Y[:], PO[:], X_cur[:], OP.mult)
                            nc.vector.tensor_tensor(
                                X_next[:, CB:], Y[:, :CB], Y[:, CB:], OP.add)
                            nc.scalar.activation(
                                TC_[:], X_next[:, CB:], AF.Tanh)
                            quad(pso, 12, 16)
                            nc.scalar.activation(O_[:], pso, AF.Sigmoid)
                            nc.vector.tensor_tensor(
                                hwin[:, :, u, :], O_[:], TC_[:], OP.mult)
                            if u == WIN - 1:
                                nc.gpsimd.tensor_copy(
                                    Hlast[:].rearrange("p (k b) -> p k b", k=KC),
                                    hwin[:, :, u, :],
                                )
                        # ---- ship h window ----
                        nc.sync.dma_start(
                            send_dram[ds(jw, 1), :, :],
                            hwin[:].rearrange("p k w b -> p (k w b)"),
                        )
                    if seg < NSEG:
                        nc.gpsimd.collective_compute(
                            "AllGather",
                            mybir.AluOpType.bypass,
                            PAIRS,
                            ins=[send_dram[:, :, :]],
                            outs=[recv_dram[:, :, :]],
                        )

            # ---- epilogue: out = relu(h_last @ Wout.T + bout) ----
            with tc.tile_pool(name="fps", bufs=2, space="PSUM") as fps_pool, \
                 tc.tile_pool(name="fo", bufs=1) as fo_pool:
                out_sb = fo_pool.tile([PB, OUT], F32)
                for half in range(2):
                    psF = fps_pool.tile([PB, 512], F32, tag="psF")
                    for kc in range(KC):
                        nc.tensor.matmul(
                            psF[:],
                            Hlast[:, kc * PB:(kc + 1) * PB],
                            wout_sb[:, kc, half * 512:(half + 1) * 512],
                            start=(kc == 0),
                            stop=(kc == KC - 1),
                        )
                    sl = slice(half * 512, (half + 1) * 512)
                    nc.vector.tensor_tensor(
                        out_sb[:, sl], psF[:], bout_sb[:, sl], OP.add
                    )
                    nc.vector.tensor_scalar_max(out_sb[:, sl], out_sb[:, sl], 0.0)
                nc.sync.dma_start(out_ext[:, :], out_sb[:])

    _split_drain_waits(nc)
    return nc


_NC_CACHE = None


def _get_nc():
    global _NC_CACHE
    if _NC_CACHE is None:
        _NC_CACHE = build_nc()
    return _NC_CACHE


def _prep_host(inputs):
    bf = ml_dtypes.bfloat16
    x = np.asarray(inputs["x"], dtype=np.float32)

    coef = 1.0 / math.gamma(0.5)
    t = np.arange(T, dtype=np.float64)
    diff = t[:, None] - t[None, :]
    W = np.where(diff > 0, (np.abs(diff) + 1e-6) ** -0.5, 0.0).astype(np.float32)
    d = (coef * W.sum(1)).astype(np.float32)
    G = (np.diag(d) - coef * W).astype(np.float32)
    GTp = np.zeros((T, TPAD), np.float32)
    GTp[:, :T] = G.T
    GT = np.ascontiguousarray(
        GTp.reshape(KC, 128, NSEG_T, WPS * WIN).transpose(2, 0, 1, 3)
    ).astype(bf)
    GT0 = np.zeros_like(GT)

    perm = np.concatenate([  # torch order i,f,g,o -> [g,i,f,o]
        np.arange(2 * H, 3 * H), np.arange(0, H),
        np.arange(H, 2 * H), np.arange(3 * H, 4 * H),
    ])

    A0 = np.zeros((G4, NP), np.float32)
    A0[:, :N] = np.asarray(inputs["Wih0"], np.float32)[perm, :N]
    A0T = np.ascontiguousarray(A0.T).astype(bf).reshape(MC, 128, G4)
    b0 = (np.asarray(inputs["bih0"], np.float32)
          + np.asarray(inputs["bhh0"], np.float32))[perm]
    b0_t = np.ascontiguousarray(b0.reshape(GC, 128).T)
    Whh0T = np.ascontiguousarray(
        np.asarray(inputs["Whh0"], np.float32)[perm].T
    ).astype(bf).reshape(KC, 128, G4)

    A1T = np.ascontiguousarray(
        np.asarray(inputs["Wih1"], np.float32)[perm].T
    ).astype(bf).reshape(KC, 128, G4)
    b1 = (np.asarray(inputs["bih1"], np.float32)
          + np.asarray(inputs["bhh1"], np.float32))[perm]
    b1_t = np.ascontiguousarray(b1.reshape(GC, 128).T)
    Whh1T = np.ascontiguousarray(
        np.asarray(inputs["Whh1"], np.float32)[perm].T
    ).astype(bf).reshape(KC, 128, G4)

    WoutT = np.ascontiguousarray(
        np.asarray(inputs["Wout"], np.float32).T
    ).astype(bf).reshape(KC, 128, OUT)
    bout_r = np.broadcast_to(
        np.asarray(inputs["bout"], np.float32), (PB, OUT)
    ).copy()

    xp = np.zeros((B, T, NP), np.float32)
    xp[:, :, :N] = x
    xt2 = xp.reshape(B, KC, 128, NP).astype(bf)
    xz = np.zeros((PB, KC, 128, NP), bf)

    zero_mc = np.zeros((MC, 128, G4), bf)
    zero_kc = np.zeros((KC, 128, G4), bf)
    v0_l0 = np.ones((128, 1), np.float32)
    v0_l1 = np.zeros((128, 1), np.float32)

    in_maps = []
    for c in range(NCORES):
        is_l1 = c >= NPAIR
        g = c % NPAIR
        gts = GT0 if is_l1 else GT
        m = dict(
            woutt=WoutT, boutr=bout_r,
            xt=xz if is_l1 else np.ascontiguousarray(xt2[g * PB:(g + 1) * PB]),
            amt=zero_mc if is_l1 else A0T,
            art=A1T if is_l1 else zero_kc,
            bg=b1_t if is_l1 else b0_t,
            whht=Whh1T if is_l1 else Whh0T,
            v0=v0_l1 if is_l1 else v0_l0,
        )
        for s in range(NSEG_T):
            m[f"gt{s}"] = np.ascontiguousarray(gts[s])
        in_maps.append(m)
    return in_maps


def kernel(**inputs):
    nc = _get_nc()
    in_maps = _prep_host(inputs)
    res = run_bass_kernel_spmd(nc, in_maps, core_ids=list(range(NCORES)))
    out = np.concatenate(
        [res.results[NPAIR + g]["out"] for g in range(NPAIR)], axis=0
    )
    return out.astype(np.float32)


# revision 6
# speedup vs baseline: 1.3980x; 1.0021x over previous
"""Trainium2 Bass kernel for CaputoEncoder — layer-split pipeline (v3).

Cores 0-3 run LSTM layer 0 for batch groups of 16; cores 4-7 run layer 1 for
the same groups, one segment behind, receiving h0 windows via pairwise
AllGather ({i, i+4}). One uniform SPMD program; the role difference is
expressed purely through per-core input data:

  role L0 (cores 0-3):  gt = G^T (caputo), x = x-batch, a_mine = A0,
                        a_recv = 0, whh = Whh0, bias = b0, v = ones
  role L1 (cores 4-7):  gt = 0, x = 0, a_mine = 0, a_recv = A1,
                        whh = Whh1, bias = b1, v = 0 for the first segment

Every core, per window: caputo matmuls (junk for L1), xw psum =
A_mine@feats + A_recv@recv_slot0 (+ v-scaled bias), a 32-step scan, h window
DMA'd to the send buffer. After each python-level segment, one AllGather
ships the segment's h windows; slot 0 of its output is always the L0 core's
h0, so every core reads slot 0 (L0 cores then multiply it by a_recv = 0).
L1's timeline is shifted one segment; zero xw windows keep its state exactly
zero until real data arrives, and the final linear layer (junk on L0 cores)
is taken from cores 4-7 by the host.
"""

import math
from contextlib import ExitStack

import numpy as np
import ml_dtypes

import concourse.bass as bass
import concourse.tile as tile
from concourse import mybir
from concourse.bass import ds
from concourse.bass_utils import run_bass_kernel_spmd

AF = mybir.ActivationFunctionType
OP = mybir.AluOpType
F32 = mybir.dt.float32
BF16 = mybir.dt.bfloat16

B, T, N = 64, 512, 250
H = 512
G4 = 4 * H
OUT = 1024
NCORES = 8
NPAIR = 4
PB = B // NPAIR           # 16 batches per core pair
WIN = 32
NWIN = T // WIN           # 16 real windows
NSEG = 4                  # python-level segments (CC boundaries)
WPS = NWIN // NSEG        # 4 windows per segment
NSEG_T = NSEG + 1         # +1 segment of L1 lag
TWIN = NSEG_T * WPS       # 20 total window slots
TPAD = TWIN * WIN         # 640 G^T columns
KC = H // 128
GC = G4 // 128
MC = 2
NP = MC * 128
CB = KC * PB              # 64 h/c columns
PAIRS = [[0, 4], [1, 5], [2, 6], [3, 7]]


def _split_drain_waits(nc, max_waits=1):
    for bb in nc.m.functions[0].blocks:
        insts = bb.instructions
        i = 0
        while i < len(insts):
            ins = insts[i]
            si = ins.sync_info
            if si is not None and len(si.on_wait) > max_waits:
                waits = list(si.on_wait)
                ins.sync_info = mybir.SyncInfo(
                    on_wait=waits[:max_waits], on_update=list(si.on_update)
                )
                for j, w in enumerate(waits[max_waits:]):
                    nop = mybir.InstNoOp(name=f"{ins.name}-wsplit{j}")
                    nop.engine = ins.engine
                    nop.sync_info = mybir.SyncInfo(on_wait=[w], on_update=[])
                    insts.insert(i, nop)
                    i += 1
            i += 1


def build_nc():
    nc = bass.Bass()

    x_in = nc.dram_tensor("xt", [PB, KC, 128, NP], BF16, kind="ExternalInput")
    gt_ins = [nc.dram_tensor(f"gt{s}", [KC, 128, WPS * WIN], BF16, kind="ExternalInput")
               for s in range(NSEG_T)]
    am_in = nc.dram_tensor("amt", [MC, 128, G4], BF16, kind="ExternalInput")
    ar_in = nc.dram_tensor("art", [KC, 128, G4], BF16, kind="ExternalInput")
    b_in = nc.dram_tensor("bg", [128, GC], F32, kind="ExternalInput")
    v0_in = nc.dram_tensor("v0", [128, 1], F32, kind="ExternalInput")
    whh_in = nc.dram_tensor("whht", [KC, 128, G4], BF16, kind="ExternalInput")
    wout_in = nc.dram_tensor("woutt", [KC, 128, OUT], BF16, kind="ExternalInput")
    bout_in = nc.dram_tensor("boutr", [PB, OUT], F32, kind="ExternalInput")
    out_ext = nc.dram_tensor("out", [PB, OUT], F32, kind="ExternalOutput")

    send_dram = nc.dram_tensor("sendb", [WPS, 128, KC * WIN * PB], BF16)
    recv_dram = nc.dram_tensor("recvb", [2 * WPS, 128, KC * WIN * PB], BF16)

    with tile.TileContext(nc) as tc:
        with ExitStack() as ctx:
            cpool = ctx.enter_context(tc.tile_pool(name="consts", bufs=1))
            spool = ctx.enter_context(tc.tile_pool(name="state", bufs=1))

            x_sb = cpool.tile([128, PB, KC, NP], BF16)
            nc.sync.dma_start(
                x_sb[:], x_in[:, :, :, :].rearrange("b k p n -> p b k n")
            )
            am_sb = cpool.tile([128, MC, G4], BF16)
            nc.sync.dma_start(am_sb[:], am_in[:, :, :].rearrange("k p g -> p k g"))
            ar_sb = cpool.tile([128, KC, G4], BF16)
            nc.sync.dma_start(ar_sb[:], ar_in[:, :, :].rearrange("k p g -> p k g"))
            b_sb = cpool.tile([128, GC], F32)
            nc.sync.dma_start(b_sb[:], b_in[:, :])
            whh_sb = cpool.tile([128, KC, G4], BF16)
            nc.sync.dma_start(whh_sb[:], whh_in[:, :, :].rearrange("k p g -> p k g"))
            wout_sb = cpool.tile([128, KC, OUT], BF16)
            nc.sync.dma_start(wout_sb[:], wout_in[:, :, :].rearrange("k p g -> p k g"))
            bout_sb = cpool.tile([PB, OUT], F32)
            nc.sync.dma_start(bout_sb[:], bout_in[:, :])
            v0_sb = cpool.tile([128, 1], F32)
            nc.sync.dma_start(v0_sb[:], v0_in[:, :])

            xw_sb = spool.tile([128, GC, WIN, PB], BF16)
            hwin = spool.tile([128, KC, WIN, PB], BF16)
            feats = spool.tile([128, MC, WIN, PB], BF16)
            recv_sb = spool.tile([128, KC, WIN, PB], BF16)
            Hlast = spool.tile([128, CB], BF16)
            X = [spool.tile([128, 2 * CB], F32, name=f"X_{i}") for i in range(2)]
            beff = spool.tile([128, GC], F32)
            zt = spool.tile([128, KC * WIN * PB], BF16)
            nc.vector.memset(Hlast[:], 0.0)
            nc.vector.memset(X[0][:], 0.0)
            nc.vector.memset(X[1][:], 0.0)
            nc.vector.memset(zt[:], 0.0)
            for jw in range(WPS):
                nc.sync.dma_start(recv_dram[jw, :, :], zt[:])

            with tc.tile_pool(name="gtw", bufs=2) as gtw_pool, \
                 tc.tile_pool(name="tps", bufs=2, space="PSUM") as tps_pool, \
                 tc.tile_pool(name="ps", bufs=2, space="PSUM") as ps_pool, \
                 tc.tile_pool(name="ew", bufs=3) as ew_pool:
                for seg in range(NSEG_T):
                    if seg == 0:
                        nc.gpsimd.tensor_scalar_mul(
                            beff[:], b_sb[:], v0_sb[:, 0:1])
                    elif seg == 1:
                        nc.gpsimd.tensor_copy(beff[:], b_sb[:])
                    with tc.For_i(0, WPS, 1,
                                  hint_engines=(mybir.EngineType.PE,)) as jw:
                        # ---- per-window DMAs ----
                        gtw = gtw_pool.tile([128, KC, WIN], BF16, tag="gtw")
                        nc.sync.dma_start(
                            gtw[:],
                            gt_ins[seg][:, :, ds(jw * WIN, WIN)].rearrange(
                                "k p w -> p k w"),
                        )
                        nc.sync.dma_start(
                            recv_sb[:].rearrange("p k w b -> p (k w b)"),
                            recv_dram[ds(jw, 1), :, :].rearrange(
                                "w p c -> p w c"),
                        )
                        # ---- caputo feats ----
                        for b in range(PB):
                            for mc in range(MC):
                                psC = tps_pool.tile([128, WIN], F32, tag="psC")
                                for kc in range(KC):
                                    nc.tensor.matmul(
                                        psC[:],
                                        x_sb[:, b, kc, mc * 128:(mc + 1) * 128],
                                        gtw[:, kc, :],
                                        start=(kc == 0),
                                        stop=(kc == KC - 1),
                                    )
                                nc.scalar.activation(
                                    feats[:, mc, :, b], psC[:], AF.Copy
                                )
                        # ---- xw = A_mine @ feats + A_recv @ recv + v*b ----
                        for gc in range(GC):
                            px = tps_pool.tile([128, WIN * PB], F32, tag="px")
                            for mc in range(MC):
                                nc.tensor.matmul(
                                    px[:],
                                    am_sb[:, mc, gc * 128:(gc + 1) * 128],
                                    feats[:, mc].rearrange("p w b -> p (w b)"),
                                    start=(mc == 0),
                                    stop=False,
                                )
                            for kc in range(KC):
                                nc.tensor.matmul(
                                    px[:],
                                    ar_sb[:, kc, gc * 128:(gc + 1) * 128],
                                    recv_sb[:, kc].rearrange("p w b -> p (w b)"),
                                    start=False,
                                    stop=(kc == KC - 1),
                                )
                            nc.scalar.activation(
                                xw_sb[:, gc].rearrange("p w b -> p (w b)"),
                                px[:], AF.Identity, bias=beff[:, gc:gc + 1],
                            )
                        # ---- scan 32 steps ([g,i,f,o], split psums) ----
                        for u in range(WIN):
                            pgo = ps_pool.tile([128, 2 * CB], F32, tag="pgo")
                            psif = ps_pool.tile([128, 2 * CB], F32, tag="psif")
                            psg = pgo[:, :CB]
                            pso = pgo[:, CB:]
                            nc.vector.tensor_copy(
                                psg.rearrange("p (g b) -> p g b", g=KC),
                                xw_sb[:, 0:4, u, :])
                            nc.vector.tensor_copy(
                                psif[:].rearrange("p (g b) -> p g b", g=2 * KC),
                                xw_sb[:, 4:12, u, :])
                            nc.vector.tensor_copy(
                                pso.rearrange("p (g b) -> p g b", g=KC),
                                xw_sb[:, 12:16, u, :])
                            h_aps = (
                                [Hlast[:, kc * PB:(kc + 1) * PB]
                                 for kc in range(KC)]
                                if u == 0 else
                                [hwin[:, kc, u - 1, :] for kc in range(KC)]
                            )

                            def quad(ps_t, g0, g1):
                                for gc in range(g0, g1):
                                    for kc in range(KC):
                                        nc.tensor.matmul(
                                            ps_t[:, (gc - g0) * PB:
                                                 (gc - g0 + 1) * PB],
                                            whh_sb[:, kc,
                                                   gc * 128:(gc + 1) * 128],
                                            h_aps[kc],
                                            start=False,
                                            stop=(kc == KC - 1),
                                            skip_group_check=True,
                                        )


                            PO = ew_pool.tile([128, 2 * CB], F32, tag="PO")
                            O_ = ew_pool.tile([128, CB], F32, tag="O")
                            Y = ew_pool.tile([128, 2 * CB], F32, tag="Y")
                            TC_ = ew_pool.tile([128, CB], F32, tag="TC")
                            X_cur, X_next = X[u % 2], X[(u + 1) % 2]
                            quad(psg, 0, 4)
                            nc.scalar.activation(
                                X_cur[:, :CB], psg, AF.Tanh)
                            quad(psif, 4, 12)
                            nc.scalar.activation(PO[:], psif[:], AF.Sigmoid)
                            nc.vector.tensor_tensor(
                                Y[:], PO[:], X_cur[:], OP.mult)
                            nc.vector.tensor_tensor(
                                X_next[:, CB:], Y[:, :CB], Y[:, CB:], OP.add)
                            nc.scalar.activation(
                                TC_[:], X_next[:, CB:], AF.Tanh)
                            quad(pso, 12, 16)
                            nc.scalar.activation(O_[:], pso, AF.Sigmoid)
                            nc.vector.tensor_tensor(
                                hwin[:, :, u, :], O_[:], TC_[:], OP.mult)
                            if u == WIN - 1:
                                nc.gpsimd.tensor_copy(
                                    Hlast[:].rearrange("p (k b) -> p k b", k=KC),
                                    hwin[:, :, u, :],
                                )
                        # ---- ship h window ----
                        nc.sync.dma_start(
                            send_dram[ds(jw, 1), :, :],
                            hwin[:].rearrange("p k w b -> p (k w b)"),
                        )
                    if seg < NSEG:
                        nc.gpsimd.collective_compute(
                            "AllGather",
                            mybir.AluOpType.bypass,
                            PAIRS,
                            ins=[send_dram[:, :, :]],
                            outs=[recv_dram[:, :, :]],
                        )

            # ---- epilogue: out = relu(h_last @ Wout.T + bout) ----
            with tc.tile_pool(name="fps", bufs=2, space="PSUM") as fps_pool, \
                 tc.tile_pool(name="fo", bufs=1) as fo_pool:
                out_sb = fo_pool.tile([PB, OUT], F32)
                for half in range(2):
                    psF = fps_pool.tile([PB, 512], F32, tag="psF")
                    for kc in range(KC):
                        nc.tensor.matmul(
                            psF[:],
                            Hlast[:, kc * PB:(kc + 1) * PB],
                            wout_sb[:, kc, half * 512:(half + 1) * 512],
                            start=(kc == 0),
                            stop=(kc == KC - 1),
                        )
                    sl = slice(half * 512, (half + 1) * 512)
                    nc.vector.tensor_tensor(
                        out_sb[:, sl], psF[:], bout_sb[:, sl], OP.add
                    )
                    nc.vector.tensor_scalar_max(out_sb[:, sl], out_sb[:, sl], 0.0)
                nc.sync.dma_start(out_ext[:, :], out_sb[:])

    _split_drain_waits(nc)
    return nc


_NC_CACHE = None


def _get_nc():
    global _NC_CACHE
    if _NC_CACHE is None:
        _NC_CACHE = build_nc()
    return _NC_CACHE


def _prep_host(inputs):
    bf = ml_dtypes.bfloat16
    x = np.asarray(inputs["x"], dtype=np.float32)

    coef = 1.0 / math.gamma(0.5)
    t = np.arange(T, dtype=np.float64)
    diff = t[:, None] - t[None, :]
    W = np.where(diff > 0, (np.abs(diff) + 1e-6) ** -0.5, 0.0).astype(np.float32)
    d = (coef * W.sum(1)).astype(np.float32)
    G = (np.diag(d) - coef * W).astype(np.float32)
    GTp = np.zeros((T, TPAD), np.float32)
    GTp[:, :T] = G.T
    GT = np.ascontiguousarray(
        GTp.reshape(KC, 128, NSEG_T, WPS * WIN).transpose(2, 0, 1, 3)
    ).astype(bf)
    GT0 = np.zeros_like(GT)

    perm = np.concatenate([  # torch order i,f,g,o -> [g,i,f,o]
        np.arange(2 * H, 3 * H), np.arange(0, H),
        np.arange(H, 2 * H), np.arange(3 * H, 4 * H),
    ])

    A0 = np.zeros((G4, NP), np.float32)
    A0[:, :N] = np.asarray(inputs["Wih0"], np.float32)[perm, :N]
    A0T = np.ascontiguousarray(A0.T).astype(bf).reshape(MC, 128, G4)
    b0 = (np.asarray(inputs["bih0"], np.float32)
          + np.asarray(inputs["bhh0"], np.float32))[perm]
    b0_t = np.ascontiguousarray(b0.reshape(GC, 128).T)
    Whh0T = np.ascontiguousarray(
        np.asarray(inputs["Whh0"], np.float32)[perm].T
    ).astype(bf).reshape(KC, 128, G4)

    A1T = np.ascontiguousarray(
        np.asarray(inputs["Wih1"], np.float32)[perm].T
    ).astype(bf).reshape(KC, 128, G4)
    b1 = (np.asarray(inputs["bih1"], np.float32)
          + np.asarray(inputs["bhh1"], np.float32))[perm]
    b1_t = np.ascontiguousarray(b1.reshape(GC, 128).T)
    Whh1T = np.ascontiguousarray(
        np.asarray(inputs["Whh1"], np.float32)[perm].T
    ).astype(bf).reshape(KC, 128, G4)

    WoutT = np.ascontiguousarray(
        np.asarray(inputs["Wout"], np.float32).T
    ).astype(bf).reshape(KC, 128, OUT)
    bout_r = np.broadcast_to(
        np.asarray(inputs["bout"], np.float32), (PB, OUT)
    ).copy()

    xp = np.zeros((B, T, NP), np.float32)
    xp[:, :, :N] = x
    xt2 = xp.reshape(B, KC, 128, NP).astype(bf)
    xz = np.zeros((PB, KC, 128, NP), bf)

    zero_mc = np.zeros((MC, 128, G4), bf)
    zero_kc = np.zeros((KC, 128, G4), bf)
    v0_l0 = np.ones((128, 1), np.float32)
    v0_l1 = np.zeros((128, 1), np.float32)

    in_maps = []
    for c in range(NCORES):
        is_l1 = c >= NPAIR
        g = c % NPAIR
        gts = GT0 if is_l1 else GT
        m = dict(
            woutt=WoutT, boutr=bout_r,
            xt=xz if is_l1 else np.ascontiguousarray(xt2[g * PB:(g + 1) * PB]),
            amt=zero_mc if is_l1 else A0T,
            art=A1T if is_l1 else zero_kc,
            bg=b1_t if is_l1 else b0_t,
            whht=Whh1T if is_l1 else Whh0T,
            v0=v0_l1 if is_l1 else v0_l0,
        )
        for s in range(NSEG_T):
            m[f"gt{s}"] = np.ascontiguousarray(gts[s])
        in_maps.append(m)
    return in_maps


def kernel(**inputs):
    nc = _get_nc()
    in_maps = _prep_host(inputs)
    res = run_bass_kernel_spmd(nc, in_maps, core_ids=list(range(NCORES)))
    out = np.concatenate(
        [res.results[NPAIR + g]["out"] for g in range(NPAIR)], axis=0
    )
    return out.astype(np.float32)
